# revision 75
# baseline (speedup 1.0000x reference)
"""Trainium2 Bass kernel for nn_Attention (pre-LN causal attention block).

Reference computation (B=2, T=2048, C=1024, H=16, D=64, fp32):
    xn = LayerNorm(x)                       (eps=1e-6)
    qkv = xn @ qkv_w + qkv_b;  q,k,v = split(qkv)
    scores = (q @ k^T) / sqrt(D), causal-masked, softmax
    out = (softmax @ v) reshaped @ proj_w + proj_b

Sharding (8 cores): data-parallel over B (cores 0-3 <- batch 0, 4-7 <- batch 1)
x tensor-parallel over heads (4 heads/core: qkv_w column-sharded, proj_w
row-sharded).  Each core emits a partial projection output; the host sums the
4 partials per batch and adds proj_b (the "all-reduce after proj" done
host-side).

Device kernel design notes (v2, pipelined; cost-model est ~179us/core vs
~275us for v1):
  - xn is produced directly in bf16 (error budget allows: bf16 rounding of a
    ~N(0,1) activation adds ~0.4% per element, diluted through the matmuls),
    and transposed on the TENSOR engine via an identity matmul (8 x [128,128]
    blocks per tile).  A DMA-xbar transpose is NOT used: on hardware its
    completion semaphore fires early, racing with xn-tile slot reuse when the
    transposes get queue-delayed (observed whole-tile corruption).  qkv
    weights are bf16 as well, halving their HBM traffic.
  - QKV matmuls are interleaved with the LN/transpose pipeline per 4-tile
    group so the PE never waits for the whole LN phase; xnT is tile-major so
    subtile deps release each group as it lands.  x tiles are DMA'd ahead of
    the weights, V-columns of wqkv ahead of the Q/K columns.
  - Scores are computed TRANSPOSED (s^T[k,q] = K @ Q^T) so the PV matmul
    consumes softmax tiles directly with no transposes.  Softmax denominators
    come free from a ones-column appended to V (row 64 of the PV psum).
  - The denominator reciprocal is broadcast to 64 partitions with a tiny
    ones-vector matmul (no DRAM bounce), and the normalization multiply reads
    the PV psum directly, fusing the drain.
  - Attention is software-pipelined: scores(i+1) are emitted ahead of PV(i)
    so exp latency hides behind matmuls; window norms are deferred one
    iteration and the output projection of the last head is spread over the
    following iterations, with per-window output stores.
  - Causal structure skips fully-masked k/q tile combinations and trims
    partially-masked matmuls down to the fp32r full-rate minimum (N>=256).
  - fp8 was evaluated and rejected: DoubleRow QKV measures ~3.8e-2 partial
    error (over the 2e-2 budget); DoubleRow scores need a [d%32, d//32]
    partition fold whose drain cost exceeds the PE win (GPSIMD has no PSUM
    port to absorb it).
"""

import os

import numpy as np
import ml_dtypes

import concourse.bass as bass
import concourse.tile as tile
from concourse import mybir
from concourse.bass_utils import run_bass_kernel_spmd

LAST_RESULT = None
F32 = mybir.dt.float32
F32R = mybir.dt.float32r
BF16 = mybir.dt.bfloat16
FP8 = mybir.dt.float8e4

B, T, C = 2, 2048, 1024
H, D = 16, 64
HL = 4            # heads per core
CL = HL * D       # local c-dim (256)
NT = T // 128     # 16 token tiles
NCC = C // 128    # 8 contraction chunks
LN_EPS = 1e-6
SCORE_SCALE = 0.125  # 1/sqrt(D)/TEMP


def _split_waits(nc, limit=1):
    """This container's walrus rejects instructions with >1 sem wait ("Too many
    sync wait commands").  Move excess waits onto same-engine NOPs inserted
    just before the instruction (equivalent under per-engine program order)."""
    n = 0
    for f in nc.m.functions:
        for b in f.blocks:
            insts = b.instructions
            if not any(
                i.sync_info is not None and len(i.sync_info.on_wait) > limit
                for i in insts
            ):
                continue
            new = []
            for inst in insts:
                si = inst.sync_info
                if si is not None and len(si.on_wait) > limit:
                    waits = list(si.on_wait)
                    excess, keep = waits[:-limit], waits[-limit:]
                    for j in range(0, len(excess), limit):
                        n += 1
                        nop = mybir.InstNoOp(name=f"I-wsplit-{n}", ins=[], outs=[])
                        nop.engine = inst.engine
                        nop.sync_info = mybir.SyncInfo(
                            on_wait=excess[j : j + limit], on_update=[]
                        )
                        new.append(nop)
                    inst.sync_info = mybir.SyncInfo(
                        on_wait=keep, on_update=list(si.on_update)
                    )
                new.append(inst)
            b.instructions = new
    return n


def _build(mode, vbias, qkbias=False):
    """mode: 'causal' (tile-skip + diag mask), 'none' (no mask), 'full'
    (arbitrary mask, maskT input).  vbias/qkbias: apply qkv biases."""
    nc = bass.Bass(name="attnblk")
    x_in = nc.declare_dram_parameter("x_b", [T, C], F32, isOutput=False)
    wqkv = nc.declare_dram_parameter("wqkv", [C, 3 * CL], BF16, isOutput=False)
    wp = nc.declare_dram_parameter("wp", [CL, C], F32R, isOutput=False)
    has_bias = vbias or qkbias
    if has_bias:
        bqkv = nc.declare_dram_parameter("bqkv", [3 * CL], F32, isOutput=False)
    ident = nc.declare_dram_parameter("ident", [128, 128], BF16, isOutput=False)
    if mode == "causal":
        maskd = nc.declare_dram_parameter("maskd", [128, 128], F32R, isOutput=False)
    elif mode == "full":
        maskt = nc.declare_dram_parameter("maskt", [T, T], F32R, isOutput=False)
    y_out = nc.declare_dram_parameter("y_part", [T, C], F32, isOutput=True)
    scratch_sums = nc.dram_tensor("scratch_sums", [HL, T], F32)
    debug_stage = bool(int(os.environ.get("KERNEL_DEBUG_STAGE", "0")))
    if debug_stage:
        xnT_dbg = nc.declare_dram_parameter(
            "xnT_dbg", [128, NT * NCC * 128], BF16, isOutput=True
        )
        qT_dbg = nc.declare_dram_parameter("qT_dbg", [128, 2 * T], F32, isOutput=True)
        kT_dbg = nc.declare_dram_parameter("kT_dbg", [128, 2 * T], F32, isOutput=True)
        vp_dbg = nc.declare_dram_parameter(
            "vp_dbg", [128, NT * HL * (D + 1)], F32, isOutput=True
        )
        at_dbg = nc.declare_dram_parameter("at_dbg", [128, 2 * T], F32, isOutput=True)

    Exp = mybir.ActivationFunctionType.Exp
    Sqrt = mybir.ActivationFunctionType.Sqrt
    Ident = mybir.ActivationFunctionType.Identity
    MULT = mybir.AluOpType.mult

    with tile.TileContext(nc) as tc:
        with (
            tc.tile_pool(name="persist", bufs=1) as pp,
            tc.tile_pool(name="small", bufs=1) as pco,
            tc.tile_pool(name="xp", bufs=6) as xpool,
            tc.tile_pool(name="ln", bufs=3) as lnp,
            tc.tile_pool(name="p_pool", bufs=3) as ppool,
            tc.tile_pool(name="nrm", bufs=3) as nrmp,
            tc.tile_pool(name="out_pool", bufs=3) as outp,
            tc.tile_pool(name="m_pool", bufs=2) as mpool,
            tc.tile_pool(name="pv_ps", bufs=4, space="PSUM") as pvps,
            tc.tile_pool(name="sc_ps", bufs=4, space="PSUM") as scps,
        ):
            # ---- persistent sbuf tensors
            # xnT is tile-major so each DMA transpose writes one contiguous
            # slab (precise subtile deps: QKV matmuls start per group, not
            # after the last transpose)
            xnT = pp.tile([128, NT, NCC, 128], BF16, tag="xnT")
            qT = pp.tile([128, 2, T], F32R, tag="qT")        # pair-stacked Q^T
            kT = pp.tile([128, 2, T], F32R, tag="kT")
            Vp = pp.tile([128, NT, HL, D + 1], F32R, tag="Vp")  # V' per head
            w_qkv = pp.tile([128, NCC, 3 * CL], BF16, tag="wqkv")
            w_p = pp.tile([128, 2, C], F32R, tag="w_p")
            attnT = pp.tile([128, 2, T], F32R, tag="attnT")
            eps_t = pco.tile([128, 1], F32, tag="eps")
            ident_t = pco.tile([128, 128], BF16, tag="ident")
            ones_t = pco.tile([1, 64], F32R, tag="ones")
            if mode == "causal":
                maskd_t = pco.tile([128, 128], F32R, tag="maskd")


            nc.vector.memset(eps_t, LN_EPS)
            nc.vector.memset(ones_t.bitcast(F32), 1.0)
            # fill V' with ones; the V drains overwrite cols 0..D-1, leaving
            # each head's ones column (col D) in place.  On Pool: the 4k-elem
            # memset would delay the first LN tiles by ~4us on DVE.
            nc.gpsimd.memset(Vp[:, :, :, :].bitcast(F32), 1.0)

            # ---- DMA issue order: first group of x tiles first, then weights
            x_tiles = {}

            def fetch_x(tt):
                x_t = xpool.tile([128, C], F32, tag="x", name=f"x_{tt}")
                nc.sync.dma_start(out=x_t, in_=x_in[tt * 128 : (tt + 1) * 128, :])
                x_tiles[tt] = x_t

            for tt in range(4):
                fetch_x(tt)
            nc.sync.dma_start(out=ident_t, in_=ident[:, :])
            # V columns first (needed by the first V matmul ~6us in), then Q/K
            wq_r = wqkv.ap().rearrange("(cc p) n -> p cc n", p=128)
            nc.sync.dma_start(
                out=w_qkv[:, :, 2 * CL : 3 * CL], in_=wq_r[:, :, 2 * CL : 3 * CL]
            )
            nc.sync.dma_start(
                out=w_qkv[:, :, 0 : 2 * CL], in_=wq_r[:, :, 0 : 2 * CL]
            )
            if has_bias:
                bq_t = pco.tile([128, 2, 1], F32, tag="bq")
                bk_t = pco.tile([128, 2, 1], F32, tag="bk")
                bv_t = pco.tile([128, CL], F32, tag="bv")
                nc.sync.dma_start(
                    out=bq_t, in_=bqkv.ap()[0:CL].rearrange("(P p) -> p P", p=128)
                )
                nc.sync.dma_start(
                    out=bk_t,
                    in_=bqkv.ap()[CL : 2 * CL].rearrange("(P p) -> p P", p=128),
                )
                bv_ap = bass.AP(
                    tensor=bqkv.ap().tensor, offset=2 * CL, ap=[[0, 128], [1, CL]]
                )
                nc.sync.dma_start(out=bv_t, in_=bv_ap)

            def ln_compute(tt):
                x_t = x_tiles.pop(tt)
                stats = lnp.tile([128, 2, 6], F32, tag="stats")
                xg = x_t[:, :].rearrange("p (g d) -> p g d", g=2)
                for g2 in range(2):
                    nc.vector.bn_stats(out=stats[:, g2, :], in_=xg[:, g2, :])
                mv = lnp.tile([128, 2], F32, tag="mv")
                nc.vector.bn_aggr(out=mv[:, :], in_=stats[:, :, :])
                rstd = lnp.tile([128, 1], F32, tag="rstd")
                nc.scalar.activation(
                    out=rstd[:, :], in_=mv[:, 1:2], func=Sqrt,
                    bias=eps_t[:, :], scale=1.0,
                )
                nc.vector.reciprocal(out=rstd[:, :], in_=rstd[:, :])
                nmr = lnp.tile([128, 1], F32, tag="nmr")
                nc.vector.tensor_tensor(
                    out=nmr[:, :], in0=mv[:, 0:1], in1=rstd[:, :], op=MULT
                )
                nc.vector.tensor_scalar_mul(nmr[:, :], nmr[:, :], -1.0)
                xn_t = lnp.tile([128, C], BF16, tag="xn")
                nc.scalar.activation(
                    out=xn_t[:, :], in_=x_t[:, :], func=Ident,
                    bias=nmr[:, :], scale=rstd[:, :],
                )
                # PE transpose (via identity) instead of a DMA-xbar transpose:
                # engine-op semaphores are exact on HW, while the DMA path
                # races with xn slot reuse when transposes get queue-delayed
                tp = scps.tile([128, C], BF16, tag="ps512", name=f"tp_{tt}")
                for cc in range(NCC):
                    nc.tensor.transpose(
                        tp[:, cc * 128 : (cc + 1) * 128],
                        xn_t[:, cc * 128 : (cc + 1) * 128],
                        ident_t[:, :],
                    )
                eng = nc.vector if tt % 2 == 0 else nc.scalar
                if tt % 2 == 0:
                    nc.vector.tensor_copy(
                        out=xnT[:, tt, :, :].rearrange("p a b -> p (a b)"),
                        in_=tp[:, :],
                    )
                else:
                    nc.scalar.copy(
                        out=xnT[:, tt, :, :].rearrange("p a b -> p (a b)"),
                        in_=tp[:, :],
                    )

            def qkv_group(g):
                base = g * 512
                # V: token-major, all 4 heads at once (N=256)
                for tt in range(4 * g, 4 * g + 4):
                    ps = scps.tile([128, 512], F32, tag="ps512", name=f"vps_{tt}")
                    for cc in range(NCC):
                        nc.tensor.matmul(
                            ps[:, 0:CL],
                            xnT[:, tt, cc, :],
                            w_qkv[:, cc, 2 * CL : 3 * CL],
                            start=(cc == 0), stop=(cc == NCC - 1),
                        )
                    psh = ps[:, 0:CL].rearrange("p (h d) -> p h d", h=HL)
                    if vbias:
                        bvh = bv_t[:, :].rearrange("p (h d) -> p h d", h=HL)
                        nc.vector.tensor_tensor(
                            out=Vp[:, tt, :, 0:D], in0=psh, in1=bvh,
                            op=mybir.AluOpType.add,
                        )
                    else:
                        nc.vector.tensor_copy(out=Vp[:, tt, :, 0:D], in_=psh)
                # Q/K for this 512-token window (DoubleRow over cc pairs)
                for which, dest in ((0, qT), (1, kT)):
                    for P in range(2):
                        wcol = which * CL + P * 128
                        ps = scps.tile(
                            [128, 512], F32, tag="ps512",
                            name=f"qkps_{g}_{which}_{P}",
                        )
                        for cc in range(NCC):
                            nc.tensor.matmul(
                                ps[:, :],
                                w_qkv[:, cc, wcol : wcol + 128],
                                xnT[:, 4 * g : 4 * g + 4, cc, :],
                                start=(cc == 0), stop=(cc == NCC - 1),
                            )
                        if qkbias:
                            nc.scalar.activation(
                                out=dest[:, P, base : base + 512], in_=ps[:, :],
                                func=Ident,
                                bias=(bq_t if which == 0 else bk_t)[:, P, :],
                                scale=1.0,
                            )
                        elif which == 0:
                            nc.vector.tensor_copy(
                                out=dest[:, P, base : base + 512], in_=ps[:, :]
                            )
                        else:
                            nc.scalar.copy(
                                out=dest[:, P, base : base + 512], in_=ps[:, :]
                            )

            # ---- Phase B/D: LN + transpose + QKV, interleaved per group
            for g in range(4):
                # prefetch next group's x tiles
                for tt in range(4 * g + 4, min(4 * g + 8, NT)):
                    fetch_x(tt)
                if g == 1:
                    nc.sync.dma_start(
                        out=w_p, in_=wp.ap().rearrange("(P p) n -> p P n", p=128)
                    )
                for tt in range(4 * g, 4 * g + 4):
                    ln_compute(tt)
                qkv_group(g)
            # emitted late so its queue-slot wait can't head-of-line block the
            # x loads on the SP stream (needed only at attention start)
            if mode == "causal":
                nc.sync.dma_start(out=maskd_t, in_=maskd[:, :])

            # ---- Phase E/F/G: attention + projection, software-pipelined
            def chunks_for(i):
                if mode != "causal":
                    return [(s, 512) for s in range(0, T, 512)]
                a0 = 128 * i
                rem = a0 % 512
                flo = a0 - rem
                if rem == 0:
                    first = (a0, 512)
                elif rem == 128:
                    first = (a0, 384)
                elif rem == 256:
                    first = (a0, 256)
                else:  # rem == 384
                    first = (a0 - 128, 256)
                return [first] + [(s, 512) for s in range(flo + 512, T, 512)]

            def emit_scores(h, i):
                P, hp = h // 2, (h % 2) * 64
                a0 = 128 * i if mode == "causal" else 0
                p_t = ppool.tile([128, T], F32R, tag="p", name=f"p_{h}_{i}")
                if mode == "causal" and a0 % 512 == 384:
                    # PV's first window reads [512*(i//4), a0) as zeros
                    nc.vector.memset(
                        p_t[:, 512 * (i // 4) : a0].bitcast(F32), 0.0
                    )
                for cs, w in chunks_for(i):
                    sc = scps.tile(
                        [128, 512], F32, tag="ps512", name=f"sc_{h}_{i}_{cs}"
                    )
                    nc.tensor.matmul(
                        sc[:, 0:w],
                        kT[hp : hp + 64, P, i * 128 : (i + 1) * 128],
                        qT[hp : hp + 64, P, cs : cs + w],
                        start=True, stop=True,
                    )
                    vs = max(cs, a0)
                    nc.scalar.activation(
                        out=p_t[:, vs : cs + w], in_=sc[:, vs - cs : w],
                        func=Exp, scale=SCORE_SCALE,
                    )
                    if mode == "full":
                        m_t = mpool.tile([128, 512], F32R, tag="m")
                        nc.sync.dma_start(
                            out=m_t, in_=maskt[i * 128 : (i + 1) * 128, cs : cs + 512]
                        )
                        nc.vector.tensor_tensor(
                            out=p_t[:, cs : cs + 512], in0=p_t[:, cs : cs + 512],
                            in1=m_t[:, :], op=MULT,
                        )
                if mode == "causal":
                    nc.vector.tensor_tensor(
                        out=p_t[:, a0 : a0 + 128], in0=p_t[:, a0 : a0 + 128],
                        in1=maskd_t[:, :], op=MULT,
                    )
                return p_t

            def emit_pv(h, i, p_t, pv):
                a0 = 128 * i if mode == "causal" else 0
                j0 = i // 4 if mode == "causal" else 0
                # diag window (j0) last: it alone waits on the mask multiply
                for j in range(3, j0 - 1, -1):
                    off = 0
                    if mode == "causal" and j == j0 and (a0 - 512 * j0) in (128, 256):
                        off = a0 - 512 * j0
                    last = (i == 4 * j + 3) if mode == "causal" else (i == NT - 1)
                    nc.tensor.matmul(
                        pv[j][:, off:512],
                        Vp[:, i, h, :],
                        p_t[:, 512 * j + off : 512 * (j + 1)],
                        start=(i == 0), stop=last,
                    )

            def norm_thunk(h, j, pv_t):
                P, hp = h // 2, (h % 2) * 64
                cs = 512 * j

                def run():
                    sums = nrmp.tile([1, 512], F32R, tag="sums2", name=f"sm_{h}_{j}")
                    nc.vector.tensor_copy(out=sums[:, :], in_=pv_t[64:65, :])
                    bc = scps.tile([128, 512], F32, tag="ps512", name=f"bc_{h}_{j}")
                    nc.tensor.matmul(
                        bc[0:64, :], ones_t[:, :], sums[:, :], start=True, stop=True
                    )
                    rc = nrmp.tile([64, 512], F32, tag="rc", name=f"rc_{h}_{j}")
                    nc.vector.reciprocal(out=rc[:, :], in_=bc[0:64, :])
                    nc.vector.tensor_tensor(
                        out=attnT[hp : hp + 64, P, cs : cs + 512],
                        in0=pv_t[0:64, :], in1=rc[:, :], op=MULT,
                    )

                return run

            def proj_thunk(tt):
                def run():
                    o_t = outp.tile([128, C], F32, tag="o", name=f"o_{tt}")
                    for n in range(2):
                        ps = scps.tile(
                            [128, 512], F32, tag="ps512", name=f"pr_{tt}_{n}"
                        )
                        for P in range(2):
                            nc.tensor.matmul(
                                ps[:, :],
                                attnT[:, P, tt * 128 : (tt + 1) * 128],
                                w_p[:, P, n * 512 : (n + 1) * 512],
                                start=(P == 0), stop=(P == 1),
                            )
                        if n == 0:
                            nc.vector.tensor_copy(out=o_t[:, 0:512], in_=ps[:, :])
                        else:
                            nc.scalar.copy(out=o_t[:, 512:1024], in_=ps[:, :])
                        nc.sync.dma_start(
                            out=y_out[tt * 128 : (tt + 1) * 128,
                                      n * 512 : (n + 1) * 512],
                            in_=o_t[:, n * 512 : (n + 1) * 512],
                        )

                return run

            items = [(h, i) for h in range(HL) for i in range(NT)]
            deferred = []  # (due_tick, seq, thunk)
            seq = 0
            pend_p = {}
            pv_tiles = {}
            pend_p[items[0]] = emit_scores(*items[0])
            for idx, (h, i) in enumerate(items):
                if idx + 1 < len(items):
                    pend_p[items[idx + 1]] = emit_scores(*items[idx + 1])
                for d in sorted([d for d in deferred if d[0] <= idx]):
                    d[2]()
                    deferred.remove(d)
                if i == 0:
                    pv_tiles[h] = [
                        pvps.tile([65, 512], F32, tag="pv", name=f"pv_{h}_{jj}")
                        for jj in range(4)
                    ]
                emit_pv(h, i, pend_p.pop((h, i)), pv_tiles[h])
                for j in range(4):
                    stop = (i == 4 * j + 3) if mode == "causal" else (i == NT - 1)
                    if stop:
                        deferred.append((idx + 1, seq, norm_thunk(h, j, pv_tiles[h][j])))
                        seq += 1
                        if h == HL - 1:
                            for k in range(4):
                                deferred.append((idx + 2 + k, seq, proj_thunk(4 * j + k)))
                                seq += 1
            for d in sorted(deferred):
                d[2]()

            if debug_stage:
                nc.sync.dma_start(
                    out=xnT_dbg.ap().rearrange("p (a b c) -> p a b c", a=NT, b=NCC),
                    in_=xnT[:, :, :, :],
                )
                nc.sync.dma_start(
                    out=qT_dbg.ap().rearrange("p (a b) -> p a b", a=2),
                    in_=qT[:, :, :].bitcast(F32),
                )
                nc.sync.dma_start(
                    out=kT_dbg.ap().rearrange("p (a b) -> p a b", a=2),
                    in_=kT[:, :, :].bitcast(F32),
                )
                nc.sync.dma_start(
                    out=vp_dbg.ap().rearrange("p (a b c) -> p a b c", a=NT, b=HL),
                    in_=Vp[:, :, :, :].bitcast(F32),
                )
                nc.sync.dma_start(
                    out=at_dbg.ap().rearrange("p (a b) -> p a b", a=2),
                    in_=attnT[:, :, :].bitcast(F32),
                )

    _split_waits(nc, limit=1)
    return nc


def kernel(x, mask, ln_scale, ln_bias, qkv_w, qkv_b, proj_w, proj_b):
    x = np.ascontiguousarray(np.asarray(x), dtype=np.float32)
    mask2 = np.asarray(mask).reshape(T, T)
    ln_scale = np.asarray(ln_scale, dtype=np.float32)
    ln_bias = np.asarray(ln_bias, dtype=np.float32)
    qkv_w = np.asarray(qkv_w, dtype=np.float32)
    qkv_b = np.asarray(qkv_b, dtype=np.float32)
    proj_w = np.asarray(proj_w, dtype=np.float32)
    proj_b = np.asarray(proj_b, dtype=np.float32)

    # fold LayerNorm affine into the qkv projection (exact host-side algebra)
    w_eff = (ln_scale[:, None] * qkv_w).astype(np.float32)
    b_eff = (ln_bias @ qkv_w + qkv_b).astype(np.float32)

    if mask2.all():
        mode = "none"
    elif np.array_equal(mask2, np.tril(np.ones((T, T), dtype=mask2.dtype))):
        mode = "causal"
    else:
        mode = "full"

    in_maps = []
    core_ids = list(range(8))
    vbias = bool(np.any(b_eff[2 * C : 3 * C] != 0.0))
    qkbias = bool(np.any(b_eff[0 : 2 * C] != 0.0))
    maskt_f = None
    maskd = None
    if mode == "causal":
        maskd = np.ascontiguousarray(mask2[0:128, 0:128].T.astype(np.float32))
    elif mode == "full":
        maskt_f = np.ascontiguousarray(mask2.T.astype(np.float32))

    for core in core_ids:
        b = core // 4
        hs = 4 * (core % 4)
        cols_q = slice(hs * D, hs * D + CL)
        cols_k = slice(C + hs * D, C + hs * D + CL)
        cols_v = slice(2 * C + hs * D, 2 * C + hs * D + CL)
        wl = np.concatenate(
            [w_eff[:, cols_q], w_eff[:, cols_k], w_eff[:, cols_v]], axis=1
        )
        im = {
            "x_b": np.ascontiguousarray(x[b]),
            "wqkv": np.ascontiguousarray(wl.astype(ml_dtypes.bfloat16)),
            "wp": np.ascontiguousarray(proj_w[hs * D : hs * D + CL, :]),
            "ident": np.eye(128, dtype=ml_dtypes.bfloat16),
        }
        if vbias or qkbias:
            bl = np.concatenate([b_eff[cols_q], b_eff[cols_k], b_eff[cols_v]])
            im["bqkv"] = np.ascontiguousarray(bl)
        if mode == "causal":
            im["maskd"] = maskd
        elif mode == "full":
            im["maskt"] = maskt_f
        in_maps.append(im)

    nc = _build(mode, vbias, qkbias)
    trace = bool(int(os.environ.get("KERNEL_TRACE", "0")))
    res = run_bass_kernel_spmd(nc, in_maps, core_ids=core_ids, trace=trace)
    global LAST_RESULT
    LAST_RESULT = res

    out = np.zeros((B, T, C), dtype=np.float32)
    for core in core_ids:
        out[core // 4] += res.results[core]["y_part"]
    out += proj_b[None, None, :]
    return out


if __name__ == "__main__":
    rng = np.random.default_rng(0)
    x = rng.standard_normal((B, T, C), dtype=np.float32)
    mask = np.tril(np.ones((T, T), dtype=bool))[None, None]
    ln_scale = np.ones(C, np.float32)
    ln_bias = np.zeros(C, np.float32)
    lim = float(np.sqrt(6.0 / (C + 3 * C)))
    qkv_w = rng.uniform(-lim, lim, (C, 3 * C)).astype(np.float32)
    qkv_b = np.zeros(3 * C, np.float32)
    limp = float(np.sqrt(6.0 / (C + C)))
    proj_w = rng.uniform(-limp, limp, (C, C)).astype(np.float32)
    proj_b = np.zeros(C, np.float32)
    out = kernel(x, mask, ln_scale, ln_bias, qkv_w, qkv_b, proj_w, proj_b)
    print("out", out.shape, out.dtype, np.abs(out).max())


# revision 81
# speedup vs baseline: 1.0107x; 1.0107x over previous
"""Trainium2 Bass kernel for nn_Attention (pre-LN causal attention block).

Reference computation (B=2, T=2048, C=1024, H=16, D=64, fp32):
    xn = LayerNorm(x)                       (eps=1e-6)
    qkv = xn @ qkv_w + qkv_b;  q,k,v = split(qkv)
    scores = (q @ k^T) / sqrt(D), causal-masked, softmax
    out = (softmax @ v) reshaped @ proj_w + proj_b

Sharding (8 cores): data-parallel over B (cores 0-3 <- batch 0, 4-7 <- batch 1)
x tensor-parallel over heads (4 heads/core: qkv_w column-sharded, proj_w
row-sharded).  Each core emits a partial projection output; the host sums the
4 partials per batch and adds proj_b (the "all-reduce after proj" done
host-side).

Device kernel design notes (v2, pipelined; cost-model est ~179us/core vs
~275us for v1):
  - xn is produced directly in bf16 (error budget allows: bf16 rounding of a
    ~N(0,1) activation adds ~0.4% per element, diluted through the matmuls),
    and transposed on the TENSOR engine via an identity matmul (8 x [128,128]
    blocks per tile).  A DMA-xbar transpose is NOT used: on hardware its
    completion semaphore fires early, racing with xn-tile slot reuse when the
    transposes get queue-delayed (observed whole-tile corruption).  qkv
    weights are bf16 as well, halving their HBM traffic.
  - QKV matmuls are interleaved with the LN/transpose pipeline per 4-tile
    group so the PE never waits for the whole LN phase; xnT is tile-major so
    subtile deps release each group as it lands.  x tiles are DMA'd ahead of
    the weights, V-columns of wqkv ahead of the Q/K columns.
  - Scores are computed TRANSPOSED (s^T[k,q] = K @ Q^T) so the PV matmul
    consumes softmax tiles directly with no transposes.  Softmax denominators
    come free from a ones-column appended to V (row 64 of the PV psum).
  - The denominator reciprocal is broadcast to 64 partitions with a tiny
    ones-vector matmul (no DRAM bounce), and the normalization multiply reads
    the PV psum directly, fusing the drain.
  - Attention is software-pipelined: scores(i+1) are emitted ahead of PV(i)
    so exp latency hides behind matmuls; window norms are deferred one
    iteration and the output projection of the last head is spread over the
    following iterations, with per-window output stores.
  - Causal structure skips fully-masked k/q tile combinations and trims
    partially-masked matmuls down to the fp32r full-rate minimum (N>=256).
  - fp8 was evaluated and rejected: DoubleRow QKV measures ~3.8e-2 partial
    error (over the 2e-2 budget); DoubleRow scores need a [d%32, d//32]
    partition fold whose drain cost exceeds the PE win (GPSIMD has no PSUM
    port to absorb it).
"""

import os

import numpy as np
import ml_dtypes

import concourse.bass as bass
import concourse.tile as tile
from concourse import mybir
from concourse.bass_utils import run_bass_kernel_spmd

LAST_RESULT = None
F32 = mybir.dt.float32
F32R = mybir.dt.float32r
BF16 = mybir.dt.bfloat16
FP8 = mybir.dt.float8e4

B, T, C = 2, 2048, 1024
H, D = 16, 64
HL = 4            # heads per core
CL = HL * D       # local c-dim (256)
NT = T // 128     # 16 token tiles
NCC = C // 128    # 8 contraction chunks
LN_EPS = 1e-6
SCORE_SCALE = 0.125  # 1/sqrt(D)/TEMP


def _split_waits(nc, limit=1):
    """This container's walrus rejects instructions with >1 sem wait ("Too many
    sync wait commands").  Move excess waits onto same-engine NOPs inserted
    just before the instruction (equivalent under per-engine program order)."""
    n = 0
    for f in nc.m.functions:
        for b in f.blocks:
            insts = b.instructions
            if not any(
                i.sync_info is not None and len(i.sync_info.on_wait) > limit
                for i in insts
            ):
                continue
            new = []
            for inst in insts:
                si = inst.sync_info
                if si is not None and len(si.on_wait) > limit:
                    waits = list(si.on_wait)
                    excess, keep = waits[:-limit], waits[-limit:]
                    for j in range(0, len(excess), limit):
                        n += 1
                        nop = mybir.InstNoOp(name=f"I-wsplit-{n}", ins=[], outs=[])
                        nop.engine = inst.engine
                        nop.sync_info = mybir.SyncInfo(
                            on_wait=excess[j : j + limit], on_update=[]
                        )
                        new.append(nop)
                    inst.sync_info = mybir.SyncInfo(
                        on_wait=keep, on_update=list(si.on_update)
                    )
                new.append(inst)
            b.instructions = new
    return n


def _build(mode, vbias, qkbias=False):
    """mode: 'causal' (tile-skip + diag mask), 'none' (no mask), 'full'
    (arbitrary mask, maskT input).  vbias/qkbias: apply qkv biases."""
    nc = bass.Bass(name="attnblk")
    x_in = nc.declare_dram_parameter("x_b", [T, C], F32, isOutput=False)
    wqkv = nc.declare_dram_parameter("wqkv", [C, 3 * CL], BF16, isOutput=False)
    wp = nc.declare_dram_parameter("wp", [CL, C], F32R, isOutput=False)
    has_bias = vbias or qkbias
    if has_bias:
        bqkv = nc.declare_dram_parameter("bqkv", [3 * CL], F32, isOutput=False)
    ident = nc.declare_dram_parameter("ident", [128, 128], BF16, isOutput=False)
    if mode == "causal":
        maskd = nc.declare_dram_parameter("maskd", [128, 128], F32R, isOutput=False)
    elif mode == "full":
        maskt = nc.declare_dram_parameter("maskt", [T, T], F32R, isOutput=False)
    y_out = nc.declare_dram_parameter("y_part", [T, C], F32, isOutput=True)
    scratch_sums = nc.dram_tensor("scratch_sums", [HL, T], F32)
    debug_stage = bool(int(os.environ.get("KERNEL_DEBUG_STAGE", "0")))
    if debug_stage:
        xnT_dbg = nc.declare_dram_parameter(
            "xnT_dbg", [128, NT * NCC * 128], BF16, isOutput=True
        )
        qT_dbg = nc.declare_dram_parameter("qT_dbg", [128, 2 * T], F32, isOutput=True)
        kT_dbg = nc.declare_dram_parameter("kT_dbg", [128, 2 * T], F32, isOutput=True)
        vp_dbg = nc.declare_dram_parameter(
            "vp_dbg", [128, NT * HL * (D + 1)], F32, isOutput=True
        )
        at_dbg = nc.declare_dram_parameter("at_dbg", [128, 2 * T], F32, isOutput=True)

    Exp = mybir.ActivationFunctionType.Exp
    Sqrt = mybir.ActivationFunctionType.Sqrt
    Ident = mybir.ActivationFunctionType.Identity
    MULT = mybir.AluOpType.mult

    with tile.TileContext(nc) as tc:
        with (
            tc.tile_pool(name="persist", bufs=1) as pp,
            tc.tile_pool(name="small", bufs=1) as pco,
            tc.tile_pool(name="xp", bufs=8) as xpool,
            tc.tile_pool(name="ln", bufs=3) as lnp,
            tc.tile_pool(name="p_pool", bufs=3) as ppool,
            tc.tile_pool(name="nrm", bufs=3) as nrmp,
            tc.tile_pool(name="out_pool", bufs=3) as outp,
            tc.tile_pool(name="m_pool", bufs=2) as mpool,
            tc.tile_pool(name="pv_ps", bufs=4, space="PSUM") as pvps,
            tc.tile_pool(name="sc_ps", bufs=4, space="PSUM") as scps,
        ):
            # ---- persistent sbuf tensors
            # xnT is tile-major so each DMA transpose writes one contiguous
            # slab (precise subtile deps: QKV matmuls start per group, not
            # after the last transpose)
            xnT = pp.tile([128, NT, NCC, 128], BF16, tag="xnT")
            qT = pp.tile([128, 2, T], F32R, tag="qT")        # pair-stacked Q^T
            kT = pp.tile([128, 2, T], F32R, tag="kT")
            Vp = pp.tile([128, NT, HL, D + 1], F32R, tag="Vp")  # V' per head
            w_qkv = pp.tile([128, NCC, 3 * CL], BF16, tag="wqkv")
            w_p = pp.tile([128, 2, C], F32R, tag="w_p")
            attnT = pp.tile([128, 2, T], F32R, tag="attnT")
            eps_t = pco.tile([128, 1], F32, tag="eps")
            ident_t = pco.tile([128, 128], BF16, tag="ident")
            ones_t = pco.tile([1, 64], F32R, tag="ones")
            if mode == "causal":
                maskd_t = pco.tile([128, 128], F32R, tag="maskd")


            nc.vector.memset(eps_t, LN_EPS)
            nc.vector.memset(ones_t.bitcast(F32), 1.0)
            # fill V' with ones; the V drains overwrite cols 0..D-1, leaving
            # each head's ones column (col D) in place.  On Pool: the 4k-elem
            # memset would delay the first LN tiles by ~4us on DVE.
            nc.gpsimd.memset(Vp[:, :, :, :].bitcast(F32), 1.0)

            # ---- DMA issue order: first group of x tiles first, then weights
            x_tiles = {}

            def fetch_x(tt, split=False):
                x_t = xpool.tile([128, C], F32, tag="x", name=f"x_{tt}")
                if split:
                    # halves so the first LN stats start one transfer earlier
                    for hh in range(2):
                        nc.sync.dma_start(
                            out=x_t[:, hh * 512 : (hh + 1) * 512],
                            in_=x_in[
                                tt * 128 : (tt + 1) * 128, hh * 512 : (hh + 1) * 512
                            ],
                        )
                else:
                    nc.sync.dma_start(
                        out=x_t, in_=x_in[tt * 128 : (tt + 1) * 128, :]
                    )
                x_tiles[tt] = x_t

            for tt in range(4):
                fetch_x(tt, split=(tt < 2))
            nc.sync.dma_start(out=ident_t, in_=ident[:, :])
            # V columns first (needed by the first V matmul ~6us in), then Q/K
            wq_r = wqkv.ap().rearrange("(cc p) n -> p cc n", p=128)
            nc.sync.dma_start(
                out=w_qkv[:, :, 2 * CL : 3 * CL], in_=wq_r[:, :, 2 * CL : 3 * CL]
            )
            nc.sync.dma_start(
                out=w_qkv[:, :, 0 : 2 * CL], in_=wq_r[:, :, 0 : 2 * CL]
            )
            if has_bias:
                bq_t = pco.tile([128, 2, 1], F32, tag="bq")
                bk_t = pco.tile([128, 2, 1], F32, tag="bk")
                bv_t = pco.tile([128, CL], F32, tag="bv")
                nc.sync.dma_start(
                    out=bq_t, in_=bqkv.ap()[0:CL].rearrange("(P p) -> p P", p=128)
                )
                nc.sync.dma_start(
                    out=bk_t,
                    in_=bqkv.ap()[CL : 2 * CL].rearrange("(P p) -> p P", p=128),
                )
                bv_ap = bass.AP(
                    tensor=bqkv.ap().tensor, offset=2 * CL, ap=[[0, 128], [1, CL]]
                )
                nc.sync.dma_start(out=bv_t, in_=bv_ap)

            def ln_compute(tt):
                x_t = x_tiles.pop(tt)
                stats = lnp.tile([128, 2, 6], F32, tag="stats")
                xg = x_t[:, :].rearrange("p (g d) -> p g d", g=2)
                for g2 in range(2):
                    nc.vector.bn_stats(out=stats[:, g2, :], in_=xg[:, g2, :])
                mv = lnp.tile([128, 2], F32, tag="mv")
                nc.vector.bn_aggr(out=mv[:, :], in_=stats[:, :, :])
                rstd = lnp.tile([128, 1], F32, tag="rstd")
                nc.scalar.activation(
                    out=rstd[:, :], in_=mv[:, 1:2], func=Sqrt,
                    bias=eps_t[:, :], scale=1.0,
                )
                nc.vector.reciprocal(out=rstd[:, :], in_=rstd[:, :])
                nmr = lnp.tile([128, 1], F32, tag="nmr")
                nc.vector.tensor_tensor(
                    out=nmr[:, :], in0=mv[:, 0:1], in1=rstd[:, :], op=MULT
                )
                nc.vector.tensor_scalar_mul(nmr[:, :], nmr[:, :], -1.0)
                xn_t = lnp.tile([128, C], BF16, tag="xn", bufs=5)
                nc.scalar.activation(
                    out=xn_t[:, :], in_=x_t[:, :], func=Ident,
                    bias=nmr[:, :], scale=rstd[:, :],
                )
                return xn_t

            qkv_alloc = [0]

            def qkv_ps_tile(shape, dtype, name):
                # alternate between the two psum pools: the pv pool is idle
                # during the QKV phase, doubling the effective ring so the PE
                # never waits on its own drains through slot reuse
                if qkv_alloc[0] % 2 == 0:
                    t_ = scps.tile(shape, dtype, tag="ps512", name=name)
                else:
                    t_ = pvps.tile(shape, dtype, tag="pv", name=name)
                qkv_alloc[0] += 1
                return t_

            def transpose_tile(tt, xn_t):
                # PE transpose (via identity) instead of a DMA-xbar transpose:
                # engine-op semaphores are exact on HW, while the DMA path
                # races with xn slot reuse when transposes get queue-delayed
                tp = qkv_ps_tile([128, C], BF16, f"tp_{tt}")
                for cc in range(NCC):
                    nc.tensor.transpose(
                        tp[:, cc * 128 : (cc + 1) * 128],
                        xn_t[:, cc * 128 : (cc + 1) * 128],
                        ident_t[:, :],
                    )
                eng = nc.vector if tt % 2 == 0 else nc.scalar
                if tt % 2 == 0:
                    nc.vector.tensor_copy(
                        out=xnT[:, tt, :, :].rearrange("p a b -> p (a b)"),
                        in_=tp[:, :],
                    )
                else:
                    nc.scalar.copy(
                        out=xnT[:, tt, :, :].rearrange("p a b -> p (a b)"),
                        in_=tp[:, :],
                    )

            def qkv_group(g, xns):
                base = g * 512
                for tt in range(4 * g, 4 * g + 4):
                    transpose_tile(tt, xns[tt])
                # V: token-major, all 4 heads at once (N=256)
                for tt in range(4 * g, 4 * g + 4):
                    ps = qkv_ps_tile([128, 512], F32, f"vps_{tt}")
                    for cc in range(NCC):
                        nc.tensor.matmul(
                            ps[:, 0:CL],
                            xnT[:, tt, cc, :],
                            w_qkv[:, cc, 2 * CL : 3 * CL],
                            start=(cc == 0), stop=(cc == NCC - 1),
                        )
                    psh = ps[:, 0:CL].rearrange("p (h d) -> p h d", h=HL)
                    if vbias:
                        bvh = bv_t[:, :].rearrange("p (h d) -> p h d", h=HL)
                        nc.vector.tensor_tensor(
                            out=Vp[:, tt, :, 0:D], in0=psh, in1=bvh,
                            op=mybir.AluOpType.add,
                        )
                    else:
                        nc.vector.tensor_copy(out=Vp[:, tt, :, 0:D], in_=psh)
                # Q/K for this 512-token window (DoubleRow over cc pairs)
                for which, dest in ((0, qT), (1, kT)):
                    for P in range(2):
                        wcol = which * CL + P * 128
                        ps = qkv_ps_tile(
                            [128, 512], F32, f"qkps_{g}_{which}_{P}"
                        )
                        for cc in range(NCC):
                            nc.tensor.matmul(
                                ps[:, :],
                                w_qkv[:, cc, wcol : wcol + 128],
                                xnT[:, 4 * g : 4 * g + 4, cc, :],
                                start=(cc == 0), stop=(cc == NCC - 1),
                            )
                        if qkbias:
                            nc.scalar.activation(
                                out=dest[:, P, base : base + 512], in_=ps[:, :],
                                func=Ident,
                                bias=(bq_t if which == 0 else bk_t)[:, P, :],
                                scale=1.0,
                            )
                        elif which == 0:
                            nc.vector.tensor_copy(
                                out=dest[:, P, base : base + 512], in_=ps[:, :]
                            )
                        else:
                            nc.scalar.copy(
                                out=dest[:, P, base : base + 512], in_=ps[:, :]
                            )

            # ---- Phase B/D: software-pipelined groups.  LN(g+1) is emitted
            # BEFORE group g's matmul stage so the Act/DVE streams never
            # head-of-line block xn production behind drains that wait for
            # late PE work; x is prefetched two groups ahead.
            for tt in range(4, 8):
                fetch_x(tt)
            xns = {}
            for tt in range(0, 4):
                xns[tt] = ln_compute(tt)
            for g in range(4):
                for tt in range(4 * g + 8, min(4 * g + 12, NT)):
                    fetch_x(tt)
                if g == 1:
                    nc.sync.dma_start(
                        out=w_p, in_=wp.ap().rearrange("(P p) n -> p P n", p=128)
                    )
                for tt in range(4 * g + 4, min(4 * g + 8, NT)):
                    xns[tt] = ln_compute(tt)
                qkv_group(g, xns)
            # emitted late so its queue-slot wait can't head-of-line block the
            # x loads on the SP stream (needed only at attention start)
            if mode == "causal":
                nc.sync.dma_start(out=maskd_t, in_=maskd[:, :])

            # ---- Phase E/F/G: attention + projection, software-pipelined
            def chunks_for(i):
                if mode != "causal":
                    return [(s, 512) for s in range(0, T, 512)]
                a0 = 128 * i
                rem = a0 % 512
                flo = a0 - rem
                if rem == 0:
                    first = (a0, 512)
                elif rem == 128:
                    first = (a0, 384)
                elif rem == 256:
                    first = (a0, 256)
                else:  # rem == 384
                    first = (a0 - 128, 256)
                return [first] + [(s, 512) for s in range(flo + 512, T, 512)]

            def emit_scores(h, i):
                P, hp = h // 2, (h % 2) * 64
                a0 = 128 * i if mode == "causal" else 0
                p_t = ppool.tile([128, T], F32R, tag="p", name=f"p_{h}_{i}")
                if mode == "causal" and a0 % 512 == 384:
                    # PV's first window reads [512*(i//4), a0) as zeros
                    nc.vector.memset(
                        p_t[:, 512 * (i // 4) : a0].bitcast(F32), 0.0
                    )
                for cs, w in chunks_for(i):
                    sc = scps.tile(
                        [128, 512], F32, tag="ps512", name=f"sc_{h}_{i}_{cs}"
                    )
                    nc.tensor.matmul(
                        sc[:, 0:w],
                        kT[hp : hp + 64, P, i * 128 : (i + 1) * 128],
                        qT[hp : hp + 64, P, cs : cs + w],
                        start=True, stop=True,
                    )
                    vs = max(cs, a0)
                    nc.scalar.activation(
                        out=p_t[:, vs : cs + w], in_=sc[:, vs - cs : w],
                        func=Exp, scale=SCORE_SCALE,
                    )
                    if mode == "full":
                        m_t = mpool.tile([128, 512], F32R, tag="m")
                        nc.sync.dma_start(
                            out=m_t, in_=maskt[i * 128 : (i + 1) * 128, cs : cs + 512]
                        )
                        nc.vector.tensor_tensor(
                            out=p_t[:, cs : cs + 512], in0=p_t[:, cs : cs + 512],
                            in1=m_t[:, :], op=MULT,
                        )
                if mode == "causal":
                    nc.vector.tensor_tensor(
                        out=p_t[:, a0 : a0 + 128], in0=p_t[:, a0 : a0 + 128],
                        in1=maskd_t[:, :], op=MULT,
                    )
                return p_t

            def emit_pv(h, i, p_t, pv):
                a0 = 128 * i if mode == "causal" else 0
                j0 = i // 4 if mode == "causal" else 0
                # diag window (j0) last: it alone waits on the mask multiply
                for j in range(3, j0 - 1, -1):
                    off = 0
                    if mode == "causal" and j == j0 and (a0 - 512 * j0) in (128, 256):
                        off = a0 - 512 * j0
                    last = (i == 4 * j + 3) if mode == "causal" else (i == NT - 1)
                    nc.tensor.matmul(
                        pv[j][:, off:512],
                        Vp[:, i, h, :],
                        p_t[:, 512 * j + off : 512 * (j + 1)],
                        start=(i == 0), stop=last,
                    )

            def sums_thunk(h, j, pv_t):
                # DVE-only head start: runs as soon as the pv stop lands
                sums = nrmp.tile([1, 512], F32R, tag="sums2", name=f"sm_{h}_{j}")
                nc.vector.tensor_copy(out=sums[:, :], in_=pv_t[64:65, :])
                return sums

            def norm_thunk(h, j, pv_t, sums):
                P, hp = h // 2, (h % 2) * 64
                cs = 512 * j

                def run():
                    bc = scps.tile([128, 512], F32, tag="ps512", name=f"bc_{h}_{j}")
                    nc.tensor.matmul(
                        bc[0:64, :], ones_t[:, :], sums[:, :], start=True, stop=True
                    )
                    rc = nrmp.tile([64, 512], F32, tag="rc", name=f"rc_{h}_{j}")
                    nc.vector.reciprocal(out=rc[:, :], in_=bc[0:64, :])
                    nc.vector.tensor_tensor(
                        out=attnT[hp : hp + 64, P, cs : cs + 512],
                        in0=pv_t[0:64, :], in1=rc[:, :], op=MULT,
                    )

                return run

            def proj_thunk(tt):
                def run():
                    o_t = outp.tile([128, C], F32, tag="o", name=f"o_{tt}")
                    for n in range(2):
                        ps = scps.tile(
                            [128, 512], F32, tag="ps512", name=f"pr_{tt}_{n}"
                        )
                        for P in range(2):
                            nc.tensor.matmul(
                                ps[:, :],
                                attnT[:, P, tt * 128 : (tt + 1) * 128],
                                w_p[:, P, n * 512 : (n + 1) * 512],
                                start=(P == 0), stop=(P == 1),
                            )
                        if n == 0:
                            nc.vector.tensor_copy(out=o_t[:, 0:512], in_=ps[:, :])
                        else:
                            nc.scalar.copy(out=o_t[:, 512:1024], in_=ps[:, :])
                        nc.sync.dma_start(
                            out=y_out[tt * 128 : (tt + 1) * 128,
                                      n * 512 : (n + 1) * 512],
                            in_=o_t[:, n * 512 : (n + 1) * 512],
                        )

                return run

            items = [(h, i) for h in range(HL) for i in range(NT)]
            deferred = []  # (due_tick, seq, thunk)
            seq = 0
            pend_p = {}
            pv_tiles = {}
            pend_p[items[0]] = emit_scores(*items[0])
            for idx, (h, i) in enumerate(items):
                if idx + 1 < len(items):
                    pend_p[items[idx + 1]] = emit_scores(*items[idx + 1])
                for d in sorted([d for d in deferred if d[0] <= idx]):
                    d[2]()
                    deferred.remove(d)
                if i == 0:
                    pv_tiles[h] = [
                        pvps.tile([65, 512], F32, tag="pv", name=f"pv_{h}_{jj}")
                        for jj in range(4)
                    ]
                emit_pv(h, i, pend_p.pop((h, i)), pv_tiles[h])
                for j in range(4):
                    stop = (i == 4 * j + 3) if mode == "causal" else (i == NT - 1)
                    if stop:
                        sums = sums_thunk(h, j, pv_tiles[h][j])
                        deferred.append(
                            (idx + 1, seq, norm_thunk(h, j, pv_tiles[h][j], sums))
                        )
                        seq += 1
                        if h == HL - 1:
                            for k in range(4):
                                deferred.append((idx + 2 + k, seq, proj_thunk(4 * j + k)))
                                seq += 1
            for d in sorted(deferred):
                d[2]()

            if debug_stage:
                nc.sync.dma_start(
                    out=xnT_dbg.ap().rearrange("p (a b c) -> p a b c", a=NT, b=NCC),
                    in_=xnT[:, :, :, :],
                )
                nc.sync.dma_start(
                    out=qT_dbg.ap().rearrange("p (a b) -> p a b", a=2),
                    in_=qT[:, :, :].bitcast(F32),
                )
                nc.sync.dma_start(
                    out=kT_dbg.ap().rearrange("p (a b) -> p a b", a=2),
                    in_=kT[:, :, :].bitcast(F32),
                )
                nc.sync.dma_start(
                    out=vp_dbg.ap().rearrange("p (a b c) -> p a b c", a=NT, b=HL),
                    in_=Vp[:, :, :, :].bitcast(F32),
                )
                nc.sync.dma_start(
                    out=at_dbg.ap().rearrange("p (a b) -> p a b", a=2),
                    in_=attnT[:, :, :].bitcast(F32),
                )

    _split_waits(nc, limit=1)
    return nc


def kernel(x, mask, ln_scale, ln_bias, qkv_w, qkv_b, proj_w, proj_b):
    x = np.ascontiguousarray(np.asarray(x), dtype=np.float32)
    mask2 = np.asarray(mask).reshape(T, T)
    ln_scale = np.asarray(ln_scale, dtype=np.float32)
    ln_bias = np.asarray(ln_bias, dtype=np.float32)
    qkv_w = np.asarray(qkv_w, dtype=np.float32)
    qkv_b = np.asarray(qkv_b, dtype=np.float32)
    proj_w = np.asarray(proj_w, dtype=np.float32)
    proj_b = np.asarray(proj_b, dtype=np.float32)

    # fold LayerNorm affine into the qkv projection (exact host-side algebra)
    w_eff = (ln_scale[:, None] * qkv_w).astype(np.float32)
    b_eff = (ln_bias @ qkv_w + qkv_b).astype(np.float32)

    if mask2.all():
        mode = "none"
    elif np.array_equal(mask2, np.tril(np.ones((T, T), dtype=mask2.dtype))):
        mode = "causal"
    else:
        mode = "full"

    in_maps = []
    core_ids = list(range(8))
    vbias = bool(np.any(b_eff[2 * C : 3 * C] != 0.0))
    qkbias = bool(np.any(b_eff[0 : 2 * C] != 0.0))
    maskt_f = None
    maskd = None
    if mode == "causal":
        maskd = np.ascontiguousarray(mask2[0:128, 0:128].T.astype(np.float32))
    elif mode == "full":
        maskt_f = np.ascontiguousarray(mask2.T.astype(np.float32))

    for core in core_ids:
        b = core // 4
        hs = 4 * (core % 4)
        cols_q = slice(hs * D, hs * D + CL)
        cols_k = slice(C + hs * D, C + hs * D + CL)
        cols_v = slice(2 * C + hs * D, 2 * C + hs * D + CL)
        wl = np.concatenate(
            [w_eff[:, cols_q], w_eff[:, cols_k], w_eff[:, cols_v]], axis=1
        )
        im = {
            "x_b": np.ascontiguousarray(x[b]),
            "wqkv": np.ascontiguousarray(wl.astype(ml_dtypes.bfloat16)),
            "wp": np.ascontiguousarray(proj_w[hs * D : hs * D + CL, :]),
            "ident": np.eye(128, dtype=ml_dtypes.bfloat16),
        }
        if vbias or qkbias:
            bl = np.concatenate([b_eff[cols_q], b_eff[cols_k], b_eff[cols_v]])
            im["bqkv"] = np.ascontiguousarray(bl)
        if mode == "causal":
            im["maskd"] = maskd
        elif mode == "full":
            im["maskt"] = maskt_f
        in_maps.append(im)

    nc = _build(mode, vbias, qkbias)
    trace = bool(int(os.environ.get("KERNEL_TRACE", "0")))
    res = run_bass_kernel_spmd(nc, in_maps, core_ids=core_ids, trace=trace)
    global LAST_RESULT
    LAST_RESULT = res

    out = np.zeros((B, T, C), dtype=np.float32)
    for core in core_ids:
        out[core // 4] += res.results[core]["y_part"]
    out += proj_b[None, None, :]
    return out


if __name__ == "__main__":
    rng = np.random.default_rng(0)
    x = rng.standard_normal((B, T, C), dtype=np.float32)
    mask = np.tril(np.ones((T, T), dtype=bool))[None, None]
    ln_scale = np.ones(C, np.float32)
    ln_bias = np.zeros(C, np.float32)
    lim = float(np.sqrt(6.0 / (C + 3 * C)))
    qkv_w = rng.uniform(-lim, lim, (C, 3 * C)).astype(np.float32)
    qkv_b = np.zeros(3 * C, np.float32)
    limp = float(np.sqrt(6.0 / (C + C)))
    proj_w = rng.uniform(-limp, limp, (C, C)).astype(np.float32)
    proj_b = np.zeros(C, np.float32)
    out = kernel(x, mask, ln_scale, ln_bias, qkv_w, qkv_b, proj_w, proj_b)
    print("out", out.shape, out.dtype, np.abs(out).max())
